# revision 9
# baseline (speedup 1.0000x reference)
"""Multi-head cross-attention (B=2, N=1024, L=4096, D=1024, H=16) on 8 trn2
NeuronCores — bf16 v2.

Sharding: batch x head-group data/tensor parallel. Core c handles batch
c//4 and heads 4*(c%4) .. 4*(c%4)+3 (weight columns sliced per head group,
Wo row-sliced; partial outputs summed on the host during unsharding).

v2 changes vs the fp32r baseline:
  - all matmul operands bf16 (fp32 PSUM accumulate): fp32 moving operands
    stream at 2 cycles/col on the PE xbus, bf16 at 1 — halves matmul time
    and DMA traffic, and the DMA'd bf16 tiles feed matmuls directly (no
    fp32->fp32r DVE casts).
  - padding mask applied as a per-key additive bias (-60) inside the exp
    activation (bias is a [128,1] per-partition AP), so V needs no keep
    premultiply; the augmented-V ones column provides the denominator.
  - q/k biases folded into the PSUM->SBUF copies (tensor_scalar_add with a
    per-partition bias vector); v bias added during the va build.
  - AV matmuls for key-block kb are dripped into kb+1's projection matmuls
    so the PE never stalls on the exp (ACT) latency.
"""
import sys

sys.path.insert(0, "/opt/trn_rl_repo")

import numpy as np

import concourse.bass as bass
import concourse.tile as tile
from concourse import bacc, mybir
from concourse.bass_utils import run_bass_kernel_spmd

dt = mybir.dt
ts = bass.ts

B, N, L, D = 2, 1024, 4096, 1024
H, DH = 16, 64
HC = 4            # heads per core
CS = HC * DH      # 256 channel slice per core
SCALE = DH ** -0.5
N_CORES = 8
QB, KB = 2, 8     # query blocks of 512, key blocks of 512
DQC = 8           # contraction chunks of 128
KT = 32           # keytiles of 128
MASK_BIAS = -60.0

TRACE = False
LAST_EXEC_NS = None
_cache = {}


def _build():
    nc = bacc.Bacc("TRN2", target_bir_lowering=False, debug=False,
                   num_devices=N_CORES)
    bf = dt.bfloat16

    xTq = nc.dram_tensor("xTq", [D, N], bf, kind="ExternalInput").ap()
    xTkv = nc.dram_tensor("xTkv", [D, L], bf, kind="ExternalInput").ap()
    # weights pre-chunked on the host to [128, chunk, F] so the DMA reads
    # contiguous 4KB-per-partition lines instead of 512B strided rows
    wq = nc.dram_tensor("wq", [128, DQC, CS], bf, kind="ExternalInput").ap()
    wk = nc.dram_tensor("wk", [128, DQC, CS], bf, kind="ExternalInput").ap()
    wv = nc.dram_tensor("wv", [128, DQC, CS], bf, kind="ExternalInput").ap()
    wo = nc.dram_tensor("wo", [128, 2, D], bf, kind="ExternalInput").ap()
    bq2 = nc.dram_tensor("bq2", [128, 2], dt.float32, kind="ExternalInput").ap()
    bk2 = nc.dram_tensor("bk2", [128, 2], dt.float32, kind="ExternalInput").ap()
    bvb = nc.dram_tensor("bvb", [128, CS], dt.float32, kind="ExternalInput").ap()
    mb = nc.dram_tensor("mb", [128, KT], dt.float32, kind="ExternalInput").ap()
    out = nc.dram_tensor("out", [N, D], bf, kind="ExternalOutput").ap()

    with tile.TileContext(nc) as tc:
        _emit(nc, tc, xTq, xTkv, wq, wk, wv, wo, bq2, bk2, bvb, mb, out)
    nc.compile()
    return nc


def _emit(nc, tc, xTq, xTkv, wq, wk, wv, wo, bq2, bk2, bvb, mb, out):
    import contextlib

    bf = dt.bfloat16
    f32 = dt.float32
    ctx = contextlib.ExitStack()
    with ctx:
        persist = ctx.enter_context(tc.tile_pool(name="persist", bufs=1))
        xpool = ctx.enter_context(tc.tile_pool(name="xs", bufs=20))
        pT_pool = ctx.enter_context(tc.tile_pool(name="pT", bufs=10))
        rb_pool = ctx.enter_context(tc.tile_pool(name="rbs", bufs=2))
        outsb_pool = ctx.enter_context(tc.tile_pool(name="outsb", bufs=2))
        psT = ctx.enter_context(tc.tile_pool(name="psT", bufs=2, space="PSUM"))
        psOA_cm = tc.tile_pool(name="psOA", bufs=1, space="PSUM")
        psOA = psOA_cm.__enter__()
        lp = nc.allow_low_precision(reason="bf16 attention internals")
        lp.__enter__()

        def load_w3(name, src):
            # src: DRAM [128, d0, F] bf16 (host pre-chunked, contiguous)
            r = persist.tile(list(src.shape), bf, tag=name, name=name)
            nc.sync.dma_start(r[:], src)
            return r

        # ---- weights needed for the Q projection ------------------------
        wq_r = load_w3("wqr", wq)               # [128, 8, 256]
        bq_v = persist.tile([128, 2], f32, tag="bqv", name="bq_v")
        nc.sync.dma_start(bq_v[:], bq2)
        mb_t = persist.tile([128, KT], f32, tag="mbt", name="mb_t")
        nc.sync.dma_start(mb_t[:], mb)

        # ---- persistent activation tiles --------------------------------
        qT_sb = [persist.tile([128, N], bf, tag=f"qT{cc}", name=f"qT{cc}")
                 for cc in range(2)]
        kT_sb = [[persist.tile([128, 512], bf, tag=f"kT{cc}_{kb}",
                               name=f"kT{cc}_{kb}") for kb in range(KB)]
                 for cc in range(2)]
        va_sb = [persist.tile([128, HC, 65], bf, tag=f"va{kt}",
                              name=f"va{kt}") for kt in range(KT)]
        onT_sb = [persist.tile([128, N], bf, tag=f"onT{cc}",
                               name=f"onT{cc}") for cc in range(2)]

        # ---- Q projection ----------------------------------------------
        for qb in range(QB):
            qp = psT.tile([128, 1024], f32, tag="pp", name=f"qp{qb}")
            for dq in range(DQC):
                xf = xpool.tile([128, 512], bf, tag="xs", name=f"xfq{qb}_{dq}")
                nc.sync.dma_start(xf[:], xTq[ts(dq, 128), ts(qb, 512)])
                for cc in range(2):
                    nc.tensor.matmul(qp[:, ts(cc, 512)], wq_r[:, dq, ts(cc, 128)],
                                     xf[:], start=(dq == 0), stop=(dq == DQC - 1))
            for cc in range(2):
                nc.vector.tensor_scalar_add(qT_sb[cc][:, ts(qb, 512)],
                                            qp[:, ts(cc, 512)], bq_v[:, cc:cc + 1])

        # remaining weights (DMA priority after the q-projection inputs)
        wk_r = load_w3("wkr", wk)
        wv_r = load_w3("wvr", wv)
        wo_r = load_w3("wor", wo)               # [128, 2, 1024]
        bk_v = persist.tile([128, 2], f32, tag="bkv", name="bk_v")
        nc.sync.dma_start(bk_v[:], bk2)
        bv_b = persist.tile([128, CS], f32, tag="bvb", name="bv_b")
        nc.sync.dma_start(bv_b[:], bvb)
        bv_b3 = bv_b[:].rearrange("p (h c) -> p h c", h=HC)
        ones128 = persist.tile([1, 128], bf, tag="o128", name="ones128")
        nc.vector.memset(ones128[:], 1.0)
        # augmented-V ones column (denominator row), set once
        for kt in range(KT):
            nc.vector.memset(va_sb[kt][:, :, 64:65], 1.0)

        # ---- attention helpers ------------------------------------------
        oPs = {}

        def open_oP(qb, hp, pool, sfx):
            oPs[(qb, hp)] = [
                pool.tile([128, 512], f32, tag=f"oP{qb}{hp}{h}{sfx}",
                          name=f"oP{qb}{hp}{h}{sfx}")
                for h in range(2)
            ]

        def attn_qk(qb, hp, kt):
            kb, kti = kt // 4, kt % 4
            sp = psT.tile([128, 1024], f32, tag="pp", name=f"sp{qb}{hp}{kt}")
            for h in range(2):
                nc.tensor.matmul(
                    sp[:, ts(h, 512)],
                    kT_sb[hp][kb][ts(h, 64), ts(kti, 128)],
                    qT_sb[hp][ts(h, 64), ts(qb, 512)],
                    start=True, stop=True,
                )
            pT = pT_pool.tile([128, 1024], bf, tag="pT", name=f"pT{qb}{hp}{kt}")
            nc.scalar.activation(pT[:], sp[:], mybir.ActivationFunctionType.Exp,
                                 scale=float(SCALE), bias=mb_t[:, kt:kt + 1])
            return pT

        def attn_av(qb, hp, kt, pT):
            oP = oPs[(qb, hp)]
            for h in range(2):
                nc.tensor.matmul(
                    oP[h][0:65, :], va_sb[kt][:, hp * 2 + h, :], pT[:, ts(h, 512)],
                    start=(kt == 0), stop=(kt == KT - 1),
                )

        def attn_norm(qb, hp):
            oP = oPs.pop((qb, hp))
            rb = psT.tile([128, 1024], f32, tag="pp", name=f"rb{qb}{hp}")
            rb_sb = rb_pool.tile([128, 1024], f32, tag="rbs", name=f"rbs{qb}{hp}")
            for h in range(2):
                den = rb_pool.tile([1, 512], f32, tag="den", name=f"den{qb}{hp}{h}")
                nc.vector.tensor_copy(den[:], oP[h][64:65, :])
                rdf = rb_pool.tile([1, 512], f32, tag="rdf", name=f"rdf{qb}{hp}{h}")
                # approx_fast needs an SBUF source (PSUM source returns garbage)
                nc.vector.reciprocal_approx_fast(rdf[:], den[:])
                rd = rb_pool.tile([1, 512], bf, tag="rd", name=f"rd{qb}{hp}{h}")
                nc.vector.tensor_copy(rd[:], rdf[:])
                nc.tensor.matmul(rb[:, ts(h, 512)], ones128[:], rd[:],
                                 start=True, stop=True)
            nc.vector.tensor_copy(rb_sb[:], rb[:])
            for h in range(2):
                nc.vector.tensor_mul(onT_sb[hp][ts(h, 64), ts(qb, 512)],
                                     oP[h][0:64, :], rb_sb[0:64, ts(h, 512)])

        # ---- phase A: K/V projections + attention on hp=0 (both qb) -----
        open_oP(0, 0, psOA, "a")
        open_oP(1, 0, psOA, "a")
        pend_av = []

        def drip():
            if pend_av:
                attn_av(*pend_av.pop(0))

        for kb in range(KB):
            kp = psT.tile([128, 1024], f32, tag="pp", name=f"kp{kb}")
            xks = []
            for dq in range(DQC):
                xf = xpool.tile([128, 512], bf, tag="xs", name=f"xfk{kb}_{dq}")
                nc.sync.dma_start(xf[:], xTkv[ts(dq, 128), ts(kb, 512)])
                xks.append(xf)
                for cc in range(2):
                    nc.tensor.matmul(kp[:, ts(cc, 512)], wk_r[:, dq, ts(cc, 128)],
                                     xf[:], start=(dq == 0), stop=(dq == DQC - 1))
                drip()
            for cc in range(2):
                nc.vector.tensor_scalar_add(kT_sb[cc][kb][:], kp[:, ts(cc, 512)],
                                            bk_v[:, cc:cc + 1])

            vp = psT.tile([128, 1024], f32, tag="pp", name=f"vp{kb}")
            for dq in range(DQC):
                for t in range(4):
                    # start clears has_written for the whole 2KB psum bank, so
                    # only the first matmul touching each bank may set it
                    nc.tensor.matmul(vp[:, ts(t, 256)], xks[dq][:, ts(t, 128)],
                                     wv_r[:, dq, :],
                                     start=(dq == 0 and t % 2 == 0),
                                     stop=(dq == DQC - 1 and t % 2 == 1))
                drip()
            for t in range(4):
                kt = kb * 4 + t
                src = vp[:, ts(t, 256)].rearrange("p (h c) -> p h c", h=HC)
                nc.vector.tensor_add(va_sb[kt][:, :, 0:64], src, bv_b3)

            for t in range(4):
                kt = kb * 4 + t
                for qb in range(QB):
                    pT = attn_qk(qb, 0, kt)
                    pend_av.append((qb, 0, kt, pT))

        while pend_av:
            drip()

        # ---- phase B: attention on hp=1, one (qb) combo at a time -------
        # (0,1)'s QK/exp chain starts while the phase-A norms run on DVE;
        # its AV matmuls begin once the norms release the psOA banks.
        for kt in range(6):
            pend_av.append((0, 1, kt, attn_qk(0, 1, kt)))
        attn_norm(0, 0)
        attn_norm(1, 0)
        psOA_cm.__exit__(None, None, None)
        psOB_cm = tc.tile_pool(name="psOB", bufs=1, space="PSUM")
        psOB = psOB_cm.__enter__()
        open_oP(0, 1, psOB, "b")
        open_oP(1, 1, psOB, "b")
        for kt in range(6, KT):
            pend_av.append((0, 1, kt, attn_qk(0, 1, kt)))
            while len(pend_av) > 3:
                drip()

        def oproj(qt):
            # accumulate in the released (0,1) oP banks
            ops = []
            for eb in range(2):
                op = psOB.tile([128, 512], f32, tag=f"oP01{eb}b",
                               name=f"op{qt}_{eb}")
                ops.append(op)
                for cc in range(2):
                    nc.tensor.matmul(op[:, :], onT_sb[cc][:, ts(qt, 128)],
                                     wo_r[:, cc, ts(eb, 512)],
                                     start=(cc == 0), stop=(cc == 1))
            osb = outsb_pool.tile([128, 1024], bf, tag="osb", name=f"osb{qt}")
            for eb in range(2):
                nc.vector.tensor_copy(osb[:, ts(eb, 512)], ops[eb][:])
            nc.sync.dma_start(out[ts(qt, 128), :], osb[:])

        # (1,1) attention; (0,1)'s norm and the first half of the output
        # projection overlap its exp-bound stretch
        oproj_done = 0
        for kt in range(KT):
            pend_av.append((1, 1, kt, attn_qk(1, 1, kt)))
            while len(pend_av) > 3:
                drip()
            if kt == 2:
                attn_norm(0, 1)
            if kt >= 8 and kt % 6 == 2 and oproj_done < 4:
                oproj(oproj_done)
                oproj_done += 1
        while pend_av:
            drip()
        attn_norm(1, 1)
        for qt in range(oproj_done, 8):
            oproj(qt)

        psOB_cm.__exit__(None, None, None)
        lp.__exit__(None, None, None)


def kernel(x_q, x_kv, pad_mask, Wq, bq, Wk, bk, Wv, bv, Wo, bo):
    global LAST_EXEC_NS
    import ml_dtypes
    bf16 = ml_dtypes.bfloat16

    x_q = np.asarray(x_q, np.float32)
    x_kv = np.asarray(x_kv, np.float32)
    pad_mask = np.asarray(pad_mask)
    Wq, bq = np.asarray(Wq, np.float32), np.asarray(bq, np.float32)
    Wk, bk = np.asarray(Wk, np.float32), np.asarray(bk, np.float32)
    Wv, bv = np.asarray(Wv, np.float32), np.asarray(bv, np.float32)
    Wo, bo = np.asarray(Wo, np.float32), np.asarray(bo, np.float32)

    if "nc" not in _cache:
        _cache["nc"] = _build()
    nc = _cache["nc"]

    xTq_b = [np.ascontiguousarray(x_q[b].T.astype(bf16)) for b in range(B)]
    xTkv_b = [np.ascontiguousarray(x_kv[b].T.astype(bf16)) for b in range(B)]
    mb_b = []
    for b in range(B):
        m = np.where(pad_mask[b], np.float32(MASK_BIAS), np.float32(0.0))
        mb_b.append(np.ascontiguousarray(m.reshape(KT, 128).T.astype(np.float32)))

    in_maps = []
    for c in range(N_CORES):
        b, g = c // 4, c % 4
        hs = g * CS
        in_maps.append({
            "xTq": xTq_b[b],
            "xTkv": xTkv_b[b],
            "wq": np.ascontiguousarray(
                Wq[:, hs:hs + CS].reshape(DQC, 128, CS).transpose(1, 0, 2)
                .astype(bf16)),
            "wk": np.ascontiguousarray(
                Wk[:, hs:hs + CS].reshape(DQC, 128, CS).transpose(1, 0, 2)
                .astype(bf16)),
            "wv": np.ascontiguousarray(
                Wv[:, hs:hs + CS].reshape(DQC, 128, CS).transpose(1, 0, 2)
                .astype(bf16)),
            "wo": np.ascontiguousarray(
                Wo[hs:hs + CS, :].reshape(2, 128, D).transpose(1, 0, 2)
                .astype(bf16)),
            "bq2": np.ascontiguousarray(bq[hs:hs + CS].reshape(2, 128).T),
            "bk2": np.ascontiguousarray(bk[hs:hs + CS].reshape(2, 128).T),
            "bvb": np.ascontiguousarray(
                np.broadcast_to(bv[hs:hs + CS], (128, CS)).astype(np.float32)),
            "mb": mb_b[b],
        })

    res = run_bass_kernel_spmd(nc, in_maps, list(range(N_CORES)), trace=TRACE)
    LAST_EXEC_NS = res.exec_time_ns

    outp = np.zeros((B, N, D), np.float32)
    for c in range(N_CORES):
        outp[c // 4] += res.results[c]["out"].astype(np.float32)
    outp += bo
    return outp


# revision 12
# speedup vs baseline: 1.1655x; 1.1655x over previous
"""Multi-head cross-attention (B=2, N=1024, L=4096, D=1024, H=16) on 8 trn2
NeuronCores — bf16 v2.

Sharding: batch x head-group data/tensor parallel. Core c handles batch
c//4 and heads 4*(c%4) .. 4*(c%4)+3 (weight columns sliced per head group,
Wo row-sliced; partial outputs summed on the host during unsharding).

v2 changes vs the fp32r baseline:
  - all matmul operands bf16 (fp32 PSUM accumulate): fp32 moving operands
    stream at 2 cycles/col on the PE xbus, bf16 at 1 — halves matmul time
    and DMA traffic, and the DMA'd bf16 tiles feed matmuls directly (no
    fp32->fp32r DVE casts).
  - padding mask applied as a per-key additive bias (-60) inside the exp
    activation (bias is a [128,1] per-partition AP), so V needs no keep
    premultiply; the augmented-V ones column provides the denominator.
  - q/k biases folded into the PSUM->SBUF copies (tensor_scalar_add with a
    per-partition bias vector); v bias added during the va build.
  - AV matmuls for key-block kb are dripped into kb+1's projection matmuls
    so the PE never stalls on the exp (ACT) latency.
"""
import sys

sys.path.insert(0, "/opt/trn_rl_repo")

import numpy as np

import concourse.bass as bass
import concourse.tile as tile
from concourse import bacc, mybir
from concourse.bass_utils import run_bass_kernel_spmd

dt = mybir.dt
ts = bass.ts

B, N, L, D = 2, 1024, 4096, 1024
H, DH = 16, 64
HC = 4            # heads per core
CS = HC * DH      # 256 channel slice per core
SCALE = DH ** -0.5
N_CORES = 8
QB, KB = 2, 8     # query blocks of 512, key blocks of 512
DQC = 8           # contraction chunks of 128
KT = 32           # keytiles of 128
MASK_BIAS = -60.0

TRACE = False
LAST_EXEC_NS = None
_cache = {}


def _build():
    nc = bacc.Bacc("TRN2", target_bir_lowering=False, debug=False,
                   num_devices=N_CORES)
    bf = dt.bfloat16

    xTq = nc.dram_tensor("xTq", [D, N], bf, kind="ExternalInput").ap()
    xTkv = nc.dram_tensor("xTkv", [D, L], bf, kind="ExternalInput").ap()
    # weights pre-chunked on the host to [128, chunk, F] so the DMA reads
    # contiguous 4KB-per-partition lines instead of 512B strided rows
    wq = nc.dram_tensor("wq", [128, DQC, CS], bf, kind="ExternalInput").ap()
    wk = nc.dram_tensor("wk", [128, DQC, CS], bf, kind="ExternalInput").ap()
    wv = nc.dram_tensor("wv", [128, DQC, CS], bf, kind="ExternalInput").ap()
    wo = nc.dram_tensor("wo", [128, 2, D], bf, kind="ExternalInput").ap()
    bq2 = nc.dram_tensor("bq2", [128, 2], dt.float32, kind="ExternalInput").ap()
    bk2 = nc.dram_tensor("bk2", [128, 2], dt.float32, kind="ExternalInput").ap()
    bvb = nc.dram_tensor("bvb", [128, CS], dt.float32, kind="ExternalInput").ap()
    mb = nc.dram_tensor("mb", [128, KT], dt.float32, kind="ExternalInput").ap()
    out = nc.dram_tensor("out", [N, D], bf, kind="ExternalOutput").ap()

    with tile.TileContext(nc) as tc:
        _emit(nc, tc, xTq, xTkv, wq, wk, wv, wo, bq2, bk2, bvb, mb, out)
    nc.compile()
    return nc


def _emit(nc, tc, xTq, xTkv, wq, wk, wv, wo, bq2, bk2, bvb, mb, out):
    import contextlib

    bf = dt.bfloat16
    f32 = dt.float32
    ctx = contextlib.ExitStack()
    with ctx:
        persist = ctx.enter_context(tc.tile_pool(name="persist", bufs=1))
        xpool = ctx.enter_context(tc.tile_pool(name="xs", bufs=12))
        pT_pool = ctx.enter_context(tc.tile_pool(name="pT", bufs=10))
        rb_pool = ctx.enter_context(tc.tile_pool(name="rbs", bufs=2))
        outsb_pool = ctx.enter_context(tc.tile_pool(name="outsb", bufs=2))
        psT = ctx.enter_context(tc.tile_pool(name="psT", bufs=2, space="PSUM"))
        psOA_cm = tc.tile_pool(name="psOA", bufs=1, space="PSUM")
        psOA = psOA_cm.__enter__()
        lp = nc.allow_low_precision(reason="bf16 attention internals")
        lp.__enter__()

        def load_w3(name, src):
            # src: DRAM [128, d0, F] bf16 (host pre-chunked, contiguous).
            # Weights ride the ACT hardware DMA queue so they stream in
            # parallel with the x DMAs on the SP queue.
            r = persist.tile(list(src.shape), bf, tag=name, name=name)
            nc.scalar.dma_start(r[:], src)
            return r

        # ---- weights needed for the Q projection ------------------------
        wq_r = load_w3("wqr", wq)               # [128, 8, 256]
        bq_v = persist.tile([128, 2], f32, tag="bqv", name="bq_v")
        nc.scalar.dma_start(bq_v[:], bq2)
        mb_t = persist.tile([128, KT], f32, tag="mbt", name="mb_t")
        nc.scalar.dma_start(mb_t[:], mb)

        # ---- persistent activation tiles --------------------------------
        qT_sb = [persist.tile([128, N], bf, tag=f"qT{cc}", name=f"qT{cc}")
                 for cc in range(2)]
        kT_sb = [[persist.tile([128, 512], bf, tag=f"kT{cc}_{kb}",
                               name=f"kT{cc}_{kb}") for kb in range(KB)]
                 for cc in range(2)]
        va_sb = [persist.tile([128, HC, 65], bf, tag=f"va{kt}",
                              name=f"va{kt}") for kt in range(KT)]
        onT_sb = [persist.tile([128, N], bf, tag=f"onT{cc}",
                               name=f"onT{cc}") for cc in range(2)]

        # ---- Q projection ----------------------------------------------
        for qb in range(QB):
            qp = psT.tile([128, 1024], f32, tag="pp", name=f"qp{qb}")
            for dq in range(DQC):
                xf = xpool.tile([128, 512], bf, tag="xs", name=f"xfq{qb}_{dq}")
                nc.sync.dma_start(xf[:], xTq[ts(dq, 128), ts(qb, 512)])
                for cc in range(2):
                    nc.tensor.matmul(qp[:, ts(cc, 512)], wq_r[:, dq, ts(cc, 128)],
                                     xf[:], start=(dq == 0), stop=(dq == DQC - 1))
            for cc in range(2):
                nc.vector.tensor_scalar_add(qT_sb[cc][:, ts(qb, 512)],
                                            qp[:, ts(cc, 512)], bq_v[:, cc:cc + 1])

        # remaining weights (DMA priority after the q-projection inputs)
        wk_r = load_w3("wkr", wk)
        wv_r = load_w3("wvr", wv)
        wo_r = load_w3("wor", wo)               # [128, 2, 1024]
        bk_v = persist.tile([128, 2], f32, tag="bkv", name="bk_v")
        nc.scalar.dma_start(bk_v[:], bk2)
        bv_b = persist.tile([128, CS], f32, tag="bvb", name="bv_b")
        nc.scalar.dma_start(bv_b[:], bvb)
        bv_b3 = bv_b[:].rearrange("p (h c) -> p h c", h=HC)
        ones128 = persist.tile([1, 128], bf, tag="o128", name="ones128")
        nc.vector.memset(ones128[:], 1.0)
        # augmented-V ones column (denominator row), set once
        for kt in range(KT):
            nc.vector.memset(va_sb[kt][:, :, 64:65], 1.0)

        # ---- attention helpers ------------------------------------------
        oPs = {}

        def open_oP(qb, hp, pool, sfx):
            oPs[(qb, hp)] = [
                pool.tile([128, 512], f32, tag=f"oP{qb}{hp}{h}{sfx}",
                          name=f"oP{qb}{hp}{h}{sfx}")
                for h in range(2)
            ]

        def attn_qk(qb, hp, kt):
            kb, kti = kt // 4, kt % 4
            sp = psT.tile([128, 1024], f32, tag="pp", name=f"sp{qb}{hp}{kt}")
            for h in range(2):
                nc.tensor.matmul(
                    sp[:, ts(h, 512)],
                    kT_sb[hp][kb][ts(h, 64), ts(kti, 128)],
                    qT_sb[hp][ts(h, 64), ts(qb, 512)],
                    start=True, stop=True,
                )
            pT = pT_pool.tile([128, 1024], bf, tag="pT", name=f"pT{qb}{hp}{kt}")
            nc.scalar.activation(pT[:], sp[:], mybir.ActivationFunctionType.Exp,
                                 scale=float(SCALE), bias=mb_t[:, kt:kt + 1])
            return pT

        def attn_av(qb, hp, kt, pT):
            oP = oPs[(qb, hp)]
            for h in range(2):
                nc.tensor.matmul(
                    oP[h][0:65, :], va_sb[kt][:, hp * 2 + h, :], pT[:, ts(h, 512)],
                    start=(kt == 0), stop=(kt == KT - 1),
                )

        def attn_norm(qb, hp):
            oP = oPs.pop((qb, hp))
            rb = psT.tile([128, 1024], f32, tag="pp", name=f"rb{qb}{hp}")
            rb_sb = rb_pool.tile([128, 1024], f32, tag="rbs", name=f"rbs{qb}{hp}")
            for h in range(2):
                den = rb_pool.tile([1, 512], f32, tag="den", name=f"den{qb}{hp}{h}")
                nc.vector.tensor_copy(den[:], oP[h][64:65, :])
                rdf = rb_pool.tile([1, 512], f32, tag="rdf", name=f"rdf{qb}{hp}{h}")
                # approx_fast needs an SBUF source (PSUM source returns garbage)
                nc.vector.reciprocal_approx_fast(rdf[:], den[:])
                rd = rb_pool.tile([1, 512], bf, tag="rd", name=f"rd{qb}{hp}{h}")
                nc.vector.tensor_copy(rd[:], rdf[:])
                nc.tensor.matmul(rb[:, ts(h, 512)], ones128[:], rd[:],
                                 start=True, stop=True)
            nc.vector.tensor_copy(rb_sb[:], rb[:])
            for h in range(2):
                nc.vector.tensor_mul(onT_sb[hp][ts(h, 64), ts(qb, 512)],
                                     oP[h][0:64, :], rb_sb[0:64, ts(h, 512)])

        # ---- phase A: K/V projections + attention on hp=0 (both qb) -----
        open_oP(0, 0, psOA, "a")
        open_oP(1, 0, psOA, "a")
        pend_av = []

        def drip():
            if pend_av:
                attn_av(*pend_av.pop(0))

        for kb in range(KB):
            kp = psT.tile([128, 1024], f32, tag="pp", name=f"kp{kb}")
            xks = []
            for dq in range(DQC):
                xf = xpool.tile([128, 512], bf, tag="xs", name=f"xfk{kb}_{dq}")
                nc.sync.dma_start(xf[:], xTkv[ts(dq, 128), ts(kb, 512)])
                xks.append(xf)
                for cc in range(2):
                    nc.tensor.matmul(kp[:, ts(cc, 512)], wk_r[:, dq, ts(cc, 128)],
                                     xf[:], start=(dq == 0), stop=(dq == DQC - 1))
                drip()
            for cc in range(2):
                nc.vector.tensor_scalar_add(kT_sb[cc][kb][:], kp[:, ts(cc, 512)],
                                            bk_v[:, cc:cc + 1])

            vp = psT.tile([128, 1024], f32, tag="pp", name=f"vp{kb}")
            for dq in range(DQC):
                for t in range(4):
                    # start clears has_written for the whole 2KB psum bank, so
                    # only the first matmul touching each bank may set it
                    nc.tensor.matmul(vp[:, ts(t, 256)], xks[dq][:, ts(t, 128)],
                                     wv_r[:, dq, :],
                                     start=(dq == 0 and t % 2 == 0),
                                     stop=(dq == DQC - 1 and t % 2 == 1))
                drip()
            for t in range(4):
                kt = kb * 4 + t
                src = vp[:, ts(t, 256)].rearrange("p (h c) -> p h c", h=HC)
                nc.vector.tensor_add(va_sb[kt][:, :, 0:64], src, bv_b3)

            for t in range(4):
                kt = kb * 4 + t
                for qb in range(QB):
                    pT = attn_qk(qb, 0, kt)
                    pend_av.append((qb, 0, kt, pT))

        while pend_av:
            drip()

        # ---- phase B: attention on hp=1 (both qb ragged) ----------------
        # phase B's QK/exp chain starts while the phase-A norms run on DVE;
        # its AV matmuls begin once the norms release the psOA banks.
        for kt in range(3):
            for qb in range(QB):
                pend_av.append((qb, 1, kt, attn_qk(qb, 1, kt)))
        attn_norm(0, 0)
        attn_norm(1, 0)
        psOA_cm.__exit__(None, None, None)
        psOB_cm = tc.tile_pool(name="psOB", bufs=1, space="PSUM")
        psOB = psOB_cm.__enter__()
        open_oP(0, 1, psOB, "b")
        open_oP(1, 1, psOB, "b")
        for kt in range(3, KT):
            for qb in range(QB):
                pend_av.append((qb, 1, kt, attn_qk(qb, 1, kt)))
            while len(pend_av) > 4:
                drip()
        while pend_av:
            drip()

        def oproj(qt):
            # accumulate in the released (0,1) oP banks
            ops = []
            for eb in range(2):
                op = psOB.tile([128, 512], f32, tag=f"oP01{eb}b",
                               name=f"op{qt}_{eb}")
                ops.append(op)
                for cc in range(2):
                    nc.tensor.matmul(op[:, :], onT_sb[cc][:, ts(qt, 128)],
                                     wo_r[:, cc, ts(eb, 512)],
                                     start=(cc == 0), stop=(cc == 1))
            osb = outsb_pool.tile([128, 1024], bf, tag="osb", name=f"osb{qt}")
            for eb in range(2):
                nc.vector.tensor_copy(osb[:, ts(eb, 512)], ops[eb][:])
            nc.scalar.dma_start(out[ts(qt, 128), :], osb[:])

        # tail: norms feed the output projection; oproj PE work overlaps
        # the norm DVE chains
        attn_norm(0, 1)
        for qt in range(4):
            oproj(qt)
        attn_norm(1, 1)
        for qt in range(4, 8):
            oproj(qt)

        psOB_cm.__exit__(None, None, None)
        lp.__exit__(None, None, None)


def kernel(x_q, x_kv, pad_mask, Wq, bq, Wk, bk, Wv, bv, Wo, bo):
    global LAST_EXEC_NS
    import ml_dtypes
    bf16 = ml_dtypes.bfloat16

    x_q = np.asarray(x_q, np.float32)
    x_kv = np.asarray(x_kv, np.float32)
    pad_mask = np.asarray(pad_mask)
    Wq, bq = np.asarray(Wq, np.float32), np.asarray(bq, np.float32)
    Wk, bk = np.asarray(Wk, np.float32), np.asarray(bk, np.float32)
    Wv, bv = np.asarray(Wv, np.float32), np.asarray(bv, np.float32)
    Wo, bo = np.asarray(Wo, np.float32), np.asarray(bo, np.float32)

    if "nc" not in _cache:
        _cache["nc"] = _build()
    nc = _cache["nc"]

    xTq_b = [np.ascontiguousarray(x_q[b].T.astype(bf16)) for b in range(B)]
    xTkv_b = [np.ascontiguousarray(x_kv[b].T.astype(bf16)) for b in range(B)]
    mb_b = []
    for b in range(B):
        m = np.where(pad_mask[b], np.float32(MASK_BIAS), np.float32(0.0))
        mb_b.append(np.ascontiguousarray(m.reshape(KT, 128).T.astype(np.float32)))

    in_maps = []
    for c in range(N_CORES):
        b, g = c // 4, c % 4
        hs = g * CS
        in_maps.append({
            "xTq": xTq_b[b],
            "xTkv": xTkv_b[b],
            "wq": np.ascontiguousarray(
                Wq[:, hs:hs + CS].reshape(DQC, 128, CS).transpose(1, 0, 2)
                .astype(bf16)),
            "wk": np.ascontiguousarray(
                Wk[:, hs:hs + CS].reshape(DQC, 128, CS).transpose(1, 0, 2)
                .astype(bf16)),
            "wv": np.ascontiguousarray(
                Wv[:, hs:hs + CS].reshape(DQC, 128, CS).transpose(1, 0, 2)
                .astype(bf16)),
            "wo": np.ascontiguousarray(
                Wo[hs:hs + CS, :].reshape(2, 128, D).transpose(1, 0, 2)
                .astype(bf16)),
            "bq2": np.ascontiguousarray(bq[hs:hs + CS].reshape(2, 128).T),
            "bk2": np.ascontiguousarray(bk[hs:hs + CS].reshape(2, 128).T),
            "bvb": np.ascontiguousarray(
                np.broadcast_to(bv[hs:hs + CS], (128, CS)).astype(np.float32)),
            "mb": mb_b[b],
        })

    res = run_bass_kernel_spmd(nc, in_maps, list(range(N_CORES)), trace=TRACE)
    LAST_EXEC_NS = res.exec_time_ns

    outp = np.zeros((B, N, D), np.float32)
    for c in range(N_CORES):
        outp[c // 4] += res.results[c]["out"].astype(np.float32)
    outp += bo
    return outp
